# revision 44
# baseline (speedup 1.0000x reference)
"""CrossAttention Trainium2 kernel (v3).

Shapes (hardcoded from the problem spec):
  x  (32, 1024, 512) f32, xf (32, 77, 256) f32
  ln_g/ln_b (512,), tln_g/tln_b (256,)
  Wq (512,512), Wk (256,512), Wv (256,512), bq/bk/bv (512,)
  out y (32, 1024, 512) f32

Strategy (v3):
  - Data-parallel over batch: 32 batches -> 8 cores x 4 batches. No collectives.
  - Host folds LayerNorm gamma + attention scale + biases into the projection
    weights, casts to bf16, ships x natural (LN stats) and x^T (Q projection).
  - LN applied to x^T in SBUF: stats via bn_stats on natural x, per-token
    (rstd | m*rstd) rows recovered with a PE transpose + small SBUF gather DMA
    (no DRAM bounce), broadcast once per batch on gpsimd, then two vector
    tensor_tensor passes normalize x^T in place of a mean-correction matmul.
  - xf path: bn_stats layernorm -> PE transpose -> K^T and [V|1] projections.
  - S^T = k^T.T @ q^T per head (row-packed 2 heads per PE pass),
    P^T = exp(S^T) on ACT, y = P^T.T @ [V|1] with the softmax denominator in
    column 64; normalize during PSUM->SBUF (vector + gpsimd split).
  - Tensor-queue software pipeline: Q-projection matmuls of step k+1 are
    interleaved with the (LDWEIGHTS-bound) y matmuls of step k so the PE
    stays stream-bound; batch-0 input DMAs are split into small pieces so
    compute starts early.
"""

import numpy as np
import ml_dtypes

import concourse.bass as bass
import concourse.bacc as bacc
import concourse.mybir as mybir
import concourse.tile as tile
from concourse.bass_utils import run_bass_kernel_spmd
from concourse.masks import make_identity

B, T, D, N, L, H = 32, 1024, 512, 77, 256, 8
HD = D // H           # 64
NCORES = 8
BPC = B // NCORES     # 4 batches per core
EPS = 1e-5
SCALE = 1.0 / np.sqrt(HD)  # 0.125 (folded into Wq on host)

BF16 = mybir.dt.bfloat16
F32 = mybir.dt.float32

DC = D // 128         # 4 D-chunks
LC = L // 128         # 2 L-chunks
XC = T // 128         # 8 T-chunks of natural x per batch


class _Bacc(bacc.Bacc):
    """Bacc whose ACT-table chooser only finds Exp/Ln in the combined
    natural_log_exp_and_others set, so the kernel needs one table load
    instead of ping-ponging between exp_and_others and the ln set."""

    def insert_act_table_loads(self):
        import bass_rust as _br
        from concourse.hw_specs import get_activation_tables

        has_activation = any(
            isinstance(i, mybir.InstActivation)
            for blk in self.main_func.blocks
            for i in blk.instructions
        )
        if not has_activation:
            return
        pair = {
            mybir.ActivationFunctionType.Exp,
            mybir.ActivationFunctionType.Ln,
        }
        tables = []
        for name, fns in get_activation_tables(self.m.arch).items():
            if name != "natural_log_exp_and_others":
                fns = fns - pair
            tables.append((name, fns))
        _br.insert_act_table_loads(self, tables)


def _bcast_free(ap, n, pos=1):
    """Insert a stride-0 free dim of count n at position pos of an AP."""
    new_ap = list(ap.ap)
    new_ap.insert(pos, [0, n])
    return bass.AP(tensor=ap.tensor, offset=ap.offset, ap=new_ap)


def _build(bpc=BPC, has_cq=False, has_ck=False, has_cv=False):
    nc = _Bacc("TRN2", target_bir_lowering=False, debug=False)

    xn_d = nc.dram_tensor("xn", (bpc, 128, XC, D), BF16, kind="ExternalInput")
    xt_d = nc.dram_tensor("xt", (bpc, 128, DC, T), BF16, kind="ExternalInput")
    xf_d = nc.dram_tensor("xf", (bpc, N, L), BF16, kind="ExternalInput")
    wq_d = nc.dram_tensor("wq", (128, DC, D), BF16, kind="ExternalInput")
    wkv_d = nc.dram_tensor("wkv", (128, LC, 2 * D), BF16, kind="ExternalInput")
    cq_d = nc.dram_tensor("cq", (1, D), F32, kind="ExternalInput") if has_cq else None
    ck_d = nc.dram_tensor("ck", (1, D), F32, kind="ExternalInput") if has_ck else None
    cv_d = nc.dram_tensor("cv", (1, D), BF16, kind="ExternalInput") if has_cv else None
    y = nc.dram_tensor("y", (bpc, T, D), BF16, kind="ExternalOutput")

    with tile.TileContext(nc) as tc:
        _trace(tc, bpc, xn_d, xt_d, xf_d, wq_d, wkv_d, cq_d, ck_d, cv_d, y)
    nc.compile()
    return nc


def _trace(tc, bpc, xn_d, xt_d, xf_d, wq_d, wkv_d, cq_d, ck_d, cv_d, y):
    nc = tc.nc
    from contextlib import ExitStack

    ctx = ExitStack()
    with ctx:
        consts = ctx.enter_context(tc.tile_pool(name="consts", bufs=1))
        xfpool = ctx.enter_context(tc.tile_pool(name="xfpool", bufs=5))
        stats = ctx.enter_context(tc.tile_pool(name="stats", bufs=8))
        rspool = ctx.enter_context(tc.tile_pool(name="rspool", bufs=4))
        kvpool = ctx.enter_context(tc.tile_pool(name="kvpool", bufs=2))
        xpool = ctx.enter_context(tc.tile_pool(name="xpool", bufs=2))
        xtpool = ctx.enter_context(tc.tile_pool(name="xtpool", bufs=3))
        xtnpool = ctx.enter_context(tc.tile_pool(name="xtnpool", bufs=4))
        rowpool = ctx.enter_context(tc.tile_pool(name="rowpool", bufs=2))
        rbpool = ctx.enter_context(tc.tile_pool(name="rbpool", bufs=2))
        qpool = ctx.enter_context(tc.tile_pool(name="qpool", bufs=3))
        ptpool = ctx.enter_context(tc.tile_pool(name="ptpool", bufs=10))
        ypool = ctx.enter_context(tc.tile_pool(name="ypool", bufs=2))
        # PSUM: 8 banks: qps 2 (Q-proj) + stp 2 (S^T pairs + kv/stat psums)
        #                + ypsp 2x2 (y accum).
        qps = ctx.enter_context(tc.tile_pool(name="qps", bufs=2, space="PSUM"))
        stp = ctx.enter_context(tc.tile_pool(name="stp", bufs=2, space="PSUM"))
        ypsp = ctx.enter_context(tc.tile_pool(name="ypsp", bufs=2, space="PSUM"))

        # ---- constants (scalar queue), split so early pieces land fast ----
        wkv_sb = consts.tile([128, LC, 2 * D], BF16, tag="wkv")
        for j in range(2):
            nc.scalar.dma_start(
                out=wkv_sb[:, :, j * D:(j + 1) * D],
                in_=wkv_d[:, :, j * D:(j + 1) * D],
            )
        wq_sb = consts.tile([128, DC, D], BF16, tag="wq")
        for j in range(4):
            cs = slice(j * 128, (j + 1) * 128)
            nc.scalar.dma_start(out=wq_sb[:, :, cs], in_=wq_d[:, :, cs])
        eps_t = consts.tile([128, 1], F32, tag="eps")
        nc.vector.memset(eps_t, EPS)
        ones_col = consts.tile([1, 128], BF16, tag="ones_col")
        nc.vector.memset(ones_col, 1.0)
        ident = consts.tile([128, 128], BF16, tag="ident")
        make_identity(nc, ident)
        # warm the gpsimd custom-op library (~15us load latency) right away;
        # batches >= 2 use partition_broadcast, by then the lib is resident.
        warm_r = consts.tile([1, 2], BF16, tag="warm_r")
        nc.gpsimd.memset(warm_r, 0.0)
        warm_o = consts.tile([128, 2], BF16, tag="warm_o")
        nc.gpsimd.partition_broadcast(warm_o, warm_r)

        cq_sb = ck_sb = cv_sb = None
        if cq_d is not None:
            cq_sb = consts.tile([128, DC], F32, tag="cq")  # [dout_part, chunk]
            nc.gpsimd.dma_start(
                out=cq_sb, in_=cq_d.rearrange("o (c p) -> (o p) c", p=128)
            )
        if ck_d is not None:
            ck_sb = consts.tile([128, DC], F32, tag="ck")
            nc.gpsimd.dma_start(
                out=ck_sb, in_=ck_d.rearrange("o (c p) -> (o p) c", p=128)
            )
        if cv_d is not None:
            cv_sb = consts.tile([1, D], BF16, tag="cv")
            nc.gpsimd.dma_start(out=cv_sb, in_=cv_d)
            ones_row = consts.tile([1, N], BF16, tag="ones_row")
            nc.vector.memset(ones_row, 1.0)

        # ---- input DMAs (sync queue), batch 0 split for early arrival ----
        xn_t, xt_t = {}, {}
        for b in range(bpc):
            xn_t[b] = xpool.tile([128, XC, D], BF16, tag="xn", name=f"xn{b}")
            xt_t[b] = xtpool.tile([128, DC, T], BF16, tag="xt", name=f"xt{b}")
        # Batch-0-critical inputs first. Each dma_start costs ~0.65us of the
        # issuing engine's time and ~128KB/engine rides at ~22.5 B/ns, so b0
        # is cut into 128KB pieces split across the sync (xn) and scalar (xt)
        # queues; later batches are emitted after stats(0) so the row-gather
        # DMA of batch 0 isn't stuck behind them.
        xf_tiles = {}
        for b in range(bpc):
            xf_tiles[b] = xfpool.tile([N, L], BF16, tag="xf", name=f"xf{b}")
            nc.sync.dma_start(out=xf_tiles[b], in_=xf_d[b])
        for c in range(XC):
            nc.sync.dma_start(out=xn_t[0][:, c, :], in_=xn_d[0][:, c, :])
        for q in range(8):
            ts = slice(q * 128, (q + 1) * 128)
            nc.sync.dma_start(out=xt_t[0][:, :, ts], in_=xt_d[0][:, :, ts])

        def emit_late_inputs():
            # b1: quarters; b2/b3: halves (sync queue)
            for q in range(4):
                nc.sync.dma_start(
                    out=xn_t[1][:, 2 * q:2 * q + 2, :],
                    in_=xn_d[1][:, 2 * q:2 * q + 2, :],
                )
            for q in range(4):
                ts = slice(q * 256, (q + 1) * 256)
                nc.sync.dma_start(out=xt_t[1][:, :, ts], in_=xt_d[1][:, :, ts])
            for b in range(2, bpc):
                for hh in range(2):
                    nc.sync.dma_start(
                        out=xn_t[b][:, 4 * hh:4 * hh + 4, :],
                        in_=xn_d[b][:, 4 * hh:4 * hh + 4, :],
                    )
                for hh in range(2):
                    ts = slice(hh * 512, (hh + 1) * 512)
                    nc.sync.dma_start(out=xt_t[b][:, :, ts], in_=xt_d[b][:, :, ts])

        kT_b, vt_b, rb_b = {}, {}, {}
        xtn_bh, qt_s, pt_s, yh_s = {}, {}, {}, {}

        def prep_kv(b):
            """xf layernorm -> xfnT -> K^T and [V|1] projections."""
            xf_t = xf_tiles[b]
            st6 = stats.tile([N, 6], F32, tag="fst6")
            nc.vector.bn_stats(out=st6, in_=xf_t)
            mv_f = stats.tile([N, 2], F32, tag="fmv")
            nc.vector.bn_aggr(out=mv_f, in_=st6)
            rstd_f = stats.tile([N, 1], F32, tag="frstd")
            nc.scalar.activation(
                out=rstd_f, in_=mv_f[:, 1:2],
                func=mybir.ActivationFunctionType.Ln,
                bias=eps_t[:N], scale=1.0,
            )
            nc.scalar.activation(
                out=rstd_f, in_=rstd_f,
                func=mybir.ActivationFunctionType.Exp, scale=-0.5,
            )
            xfn = xfpool.tile([N, L], BF16, tag="xfn")
            nc.vector.tensor_scalar(
                out=xfn, in0=xf_t,
                scalar1=mv_f[:, 0:1], scalar2=rstd_f,
                op0=mybir.AluOpType.subtract, op1=mybir.AluOpType.mult,
            )
            xfnT = xfpool.tile([128, LC, N], BF16, tag="xfnT")
            for c in range(LC):
                tps = stp.tile([128, N], BF16, tag="sp")
                nc.tensor.transpose(
                    out=tps, in_=xfn[:, c * 128:(c + 1) * 128], identity=ident[:N, :N]
                )
                nc.scalar.copy(out=xfnT[:, c, :], in_=tps)

            kT = kvpool.tile([128, DC, N], BF16, tag="kT")
            for dc in range(DC):
                kps = stp.tile([128, N], F32, tag="sp")
                for lc in range(LC):
                    nc.tensor.matmul(
                        kps,
                        lhsT=wkv_sb[:, lc, dc * 128:(dc + 1) * 128],
                        rhs=xfnT[:, lc, :],
                        start=(lc == 0), stop=(lc == LC - 1),
                    )
                if ck_sb is not None:
                    nc.vector.tensor_scalar_add(
                        out=kps, in0=kps, scalar1=ck_sb[:, dc:dc + 1]
                    )
                nc.scalar.copy(out=kT[:, dc, :], in_=kps)
            kT_b[b] = kT

            vps = stp.tile([N, D], F32, tag="sp")
            for lc in range(LC):
                nc.tensor.matmul(
                    vps, lhsT=xfnT[:, lc, :], rhs=wkv_sb[:, lc, D:2 * D],
                    start=(lc == 0), stop=(lc == LC - 1 and cv_sb is None),
                )
            if cv_sb is not None:
                nc.tensor.matmul(vps, lhsT=ones_row, rhs=cv_sb, start=False, stop=True)
            vt = kvpool.tile([N, H, HD + 1], BF16, tag="vt")
            nc.scalar.copy(
                out=vt[:, :, 0:HD], in_=vps.rearrange("n (h d) -> n h d", h=H)
            )
            nc.vector.memset(vt[:, :, HD:HD + 1], 1.0)
            vt_b[b] = vt

        def stats_chunks(b, mvx, cs):
            # mvx is k-major [128, (m|v), XC] so later reads are contiguous
            for c in cs:
                s6 = stats.tile([128, 6], F32, tag="xst6")
                nc.vector.bn_stats(out=s6, in_=xn_t[b][:, c, :])
                nc.vector.bn_aggr(out=mvx[:, :, c], in_=s6)

        def stats_rows(b, mvx):
            """mvx [128, 2, XC] -> rb [128, (rstd | m*rstd), T] broadcast."""
            srk = stats.tile([128, 2, XC], BF16, tag="srk")
            nc.vector.tensor_copy(out=srk[:, 0, :], in_=mvx[:, 0, :])
            nc.scalar.activation(
                out=srk[:, 1, :], in_=mvx[:, 1, :].rearrange("p c -> p c ()"),
                func=mybir.ActivationFunctionType.Ln,
                bias=eps_t, scale=1.0,
            )
            nc.scalar.activation(
                out=srk[:, 1, :], in_=srk[:, 1, :],
                func=mybir.ActivationFunctionType.Exp, scale=-0.5,
            )
            tps = stp.tile([2 * XC, 128], BF16, tag="sp")
            nc.tensor.transpose(
                out=tps, in_=srk.rearrange("p k c -> p (k c)"), identity=ident
            )
            tsb = stats.tile([2 * XC, 128], BF16, tag="tsb")
            nc.scalar.copy(out=tsb, in_=tps)
            # gather rows: row2[0, k, c*128+p] = tsb[k*XC+c, p]  (token order)
            row2 = rowpool.tile([1, 2, T], BF16, tag="row2")
            # b0: issue from scalar right after the tsb copy (no queue wait);
            # later batches: gpsimd (its library load has finished by then)
            dma_eng = nc.scalar if b == 0 else nc.gpsimd
            dma_eng.dma_start(
                out=row2.rearrange("o k (c p) -> o (k c) p", p=128), in_=tsb
            )
            rb = rbpool.tile([128, 2, T], BF16, tag="rb")
            if b < 2:
                # gpsimd broadcast lib is still loading this early: broadcast
                # the (m | rstd) rows via K=1 ones-matmuls instead
                for k in range(2):
                    rbp = ypsp.tile([128, 2, 512], F32, tag="ypp", name=f"rbp{b}_{k}")
                    for j in range(2):
                        nc.tensor.matmul(
                            rbp[:, j, :], lhsT=ones_col,
                            rhs=row2[0:1, k, j * 512:(j + 1) * 512],
                            start=True, stop=True,
                        )
                    nc.scalar.copy(
                        out=rb[:, k, :],
                        in_=rbp.rearrange("p j f -> p (j f)"),
                    )
            else:
                nc.gpsimd.partition_broadcast(rb, row2)
            rb_b[b] = rb

        def norm_half(b, h):
            """xtn = xt * rstd - (m*rstd), per T-half."""
            ts = slice(h * 512, (h + 1) * 512)
            rb = rb_b[b]
            xtn = xtnpool.tile([128, DC, 512], BF16, tag="xtn")
            nc.vector.tensor_sub(
                out=xtn, in0=xt_t[b][:, :, ts], in1=_bcast_free(rb[:, 0, ts], DC)
            )
            nc.vector.tensor_mul(
                out=xtn, in0=xtn, in1=_bcast_free(rb[:, 1, ts], DC)
            )
            xtn_bh[(b, h)] = xtn

        def emit_A_dc(s, dc):
            """Q-projection for one dout chunk: 4 matmuls + psum->sbuf copy."""
            b, hf = s
            if dc == 0:
                qt_s[s] = qpool.tile([128, DC, 512], BF16, tag="qt", name=f"qt{s[0]}_{s[1]}")
            qt = qt_s[s]
            xtn = xtn_bh[(b, hf)]
            qp = qps.tile([128, 512], F32, tag="qp")
            for kc in range(DC):
                nc.tensor.matmul(
                    qp,
                    lhsT=wq_sb[:, kc, dc * 128:(dc + 1) * 128],
                    rhs=xtn[:, kc, :],
                    start=(kc == 0), stop=(kc == DC - 1),
                )
            if dc == 0:
                nc.vector.tensor_copy(out=qt[:, dc, :], in_=qp)
            else:
                nc.scalar.copy(out=qt[:, dc, :], in_=qp)
            if cq_sb is not None:
                nc.vector.tensor_scalar_add(
                    out=qt[:, dc, :], in0=qt[:, dc, :], scalar1=cq_sb[:, dc:dc + 1]
                )

        def emit_B_hp(s, hp):
            """S^T + exp -> P^T tile for one row-packed head pair."""
            b, hf = s
            kT, qt = kT_b[b], qt_s[s]
            if hp == 0:
                pt_s[s] = {}
            pt = ptpool.tile([N, 2, 512], BF16, tag="pt")
            stqs = []
            for sub in range(2):
                po = 64 * sub
                stq = stp.tile([N, 512], F32, tag="sp")
                nc.tensor.matmul(
                    stq,
                    lhsT=kT[po:po + 64, hp, :],
                    rhs=qt[po:po + 64, hp, :],
                    start=True, stop=True,
                    tile_position=(po, 0),
                )
                stqs.append(stq)
            for sub in range(2):
                nc.scalar.activation(
                    out=pt[:, sub, :], in_=stqs[sub],
                    func=mybir.ActivationFunctionType.Exp,
                )
            pt_s[s][hp] = pt

        def emit_C_ci(s, ci, split_dma=False):
            """y for one 128-token chunk: 8 matmuls, normalize, (maybe) DMA."""
            b, hf = s
            vt, pts = vt_b[b], pt_s[s]
            if ci == 0:
                yh_s[s] = ypool.tile([128, 4, D], BF16, tag="yh", name=f"yh{s[0]}_{s[1]}")
            yh = yh_s[s]
            ypp = ypsp.tile([128, 2, 512], F32, tag="ypp")
            ypv = ypp.rearrange("p j (h e) -> p j h e", h=4)
            for h in range(H):
                pt = pts[h // 2]
                nc.tensor.matmul(
                    ypv[:, h // 4, h % 4, 0:HD + 1],
                    lhsT=pt[:, h % 2, ci * 128:(ci + 1) * 128],
                    rhs=vt[:, h, :],
                    start=True, stop=True,
                )
            rs = rspool.tile([128, 2, 4], F32, tag="rs")
            nc.vector.reciprocal(out=rs, in_=ypv[:, :, :, HD:HD + 1])
            yv = yh[:, ci, :].rearrange("p (j h d) -> p j h d", j=2, h=4)
            for j in range(2):
                rs_bc = _bcast_free(rs[:, j, :], HD, pos=2)
                nc.vector.tensor_mul(out=yv[:, j], in0=ypv[:, j, :, 0:HD], in1=rs_bc)
            if split_dma:
                tok = hf * 512 + ci * 128
                engs = [nc.sync, nc.scalar, nc.gpsimd]
                for hh in range(2):
                    engs[(2 * ci + hh) % 3].dma_start(
                        out=y[b, tok + hh * 64: tok + (hh + 1) * 64, :].rearrange(
                            "(c p) d -> p c d", p=64
                        ),
                        in_=yh[hh * 64:(hh + 1) * 64, ci, :].rearrange(
                            "p d -> p () d"
                        ),
                    )

        def emit_ydma(s):
            b, hf = s
            nc.gpsimd.dma_start(
                out=y[b, hf * 512:(hf + 1) * 512].rearrange(
                    "(c p) d -> p c d", p=128
                ),
                in_=yh_s[s],
            )

        # ---- schedule ----
        steps = [(b, hf) for b in range(bpc) for hf in range(2)]

        prep_kv(0)
        mvx0 = stats.tile([128, 2, XC], F32, tag="mvx")
        stats_chunks(0, mvx0, range(XC))
        stats_rows(0, mvx0)
        norm_half(0, 0)
        emit_late_inputs()
        mvx1 = stats.tile([128, 2, XC], F32, tag="mvx")
        stats_chunks(1, mvx1, range(4))  # front half; rest during step (0,1)
        norm_half(0, 1)
        prep_kv(1)

        for dc in range(DC):
            emit_A_dc(steps[0], dc)
        for hp in range(4):
            emit_B_hp(steps[0], hp)

        mvx_pend = {}
        for i in range(1, len(steps) + 1):
            s = steps[i] if i < len(steps) else None
            prev = steps[i - 1]
            b_new = None
            if s == (0, 1):
                b_new = 1          # batch 1: back half staged during (0, 1)
                mvx_pend[1] = mvx1
            elif s is not None and s[1] == 0 and 1 <= s[0] < bpc - 1:
                b_new = s[0] + 1   # later batches staged during (b, 0)
                mvx_pend[b_new] = stats.tile([128, 2, XC], F32, tag="mvx", name=f"mvx{b_new}")
            last = s is None
            for k in range(4):
                if s is not None:
                    emit_A_dc(s, k)
                emit_C_ci(prev, k, split_dma=last)
                if b_new is not None:
                    cs = range(4 + 2 * k, 6 + 2 * k) if b_new == 1 else range(2 * k, 2 * k + 2)
                    if b_new == 1 and k >= 2:
                        cs = ()
                    stats_chunks(b_new, mvx_pend[b_new], cs)
            if not last:
                emit_ydma(prev)
                if b_new is not None:
                    if b_new >= 2:
                        prep_kv(b_new)
                    stats_rows(b_new, mvx_pend[b_new])
                    norm_half(b_new, 0)
                    norm_half(b_new, 1)
                for hp in range(4):
                    emit_B_hp(s, hp)



_CACHE = {}
TRACE = False          # set True to capture an NTFF profile on core 0
LAST_RESULTS = None    # BassKernelResults of the most recent kernel() call


def _get_nc(key):
    if key not in _CACHE:
        _CACHE[key] = _build(*key)
    return _CACHE[key]


def _lay_pcd(a, p=128):
    """[(c p), d] row-major -> [p, c, d] (partition-major SBUF layout)."""
    c = a.shape[0] // p
    return np.ascontiguousarray(a.reshape(c, p, a.shape[1]).transpose(1, 0, 2))


def kernel(x, xf, ln_g, ln_b, tln_g, tln_b, Wq, bq, Wk, bk, Wv, bv):
    x = np.asarray(x, np.float32)
    xf = np.asarray(xf, np.float32)
    bf = ml_dtypes.bfloat16
    # Fold layernorm affine + attention scale + biases into the projections.
    wq_f = np.asarray(ln_g, np.float32)[:, None] * np.asarray(Wq, np.float32) * SCALE
    cq = (np.asarray(ln_b, np.float32) @ np.asarray(Wq, np.float32)
          + np.asarray(bq, np.float32)) * SCALE
    wk_f = np.asarray(tln_g, np.float32)[:, None] * np.asarray(Wk, np.float32)
    ck = np.asarray(tln_b, np.float32) @ np.asarray(Wk, np.float32) + np.asarray(bk, np.float32)
    wv_f = np.asarray(tln_g, np.float32)[:, None] * np.asarray(Wv, np.float32)
    cv = np.asarray(tln_b, np.float32) @ np.asarray(Wv, np.float32) + np.asarray(bv, np.float32)

    has_cq = bool(np.any(cq != 0))
    has_ck = bool(np.any(ck != 0))
    has_cv = bool(np.any(cv != 0))
    nc = _get_nc((BPC, has_cq, has_ck, has_cv))

    wq_b = wq_f.astype(bf)
    wkv_b = np.concatenate([wk_f, wv_f], axis=1).astype(bf)  # [256, 1024]

    wq_lay = _lay_pcd(wq_b)                       # [128, 4, 512]
    wkv_lay = _lay_pcd(wkv_b)                     # [128, 2, 1024]

    x_b = x.astype(bf)
    xf_b = xf.astype(bf)

    in_maps = []
    for i in range(NCORES):
        xs = x_b[i * BPC:(i + 1) * BPC]
        xn_l = np.stack([_lay_pcd(xs[b]) for b in range(BPC)])            # [bpc,128,8,512]
        xt_l = np.stack([_lay_pcd(np.ascontiguousarray(xs[b].T)) for b in range(BPC)])  # [bpc,128,4,1024]
        m = {
            "xn": xn_l, "xt": xt_l,
            "xf": np.ascontiguousarray(xf_b[i * BPC:(i + 1) * BPC]),
            "wq": wq_lay, "wkv": wkv_lay,
        }
        if has_cq:
            m["cq"] = cq.reshape(1, D)
        if has_ck:
            m["ck"] = ck.reshape(1, D)
        if has_cv:
            m["cv"] = cv.reshape(1, D).astype(bf)
        in_maps.append(m)

    global LAST_RESULTS
    res = run_bass_kernel_spmd(
        nc, in_maps, core_ids=list(range(NCORES)), trace=TRACE
    )
    LAST_RESULTS = res
    out = np.concatenate([r["y"] for r in res.results], axis=0)
    return out.astype(np.float32)
